# revision 51
# baseline (speedup 1.0000x reference)
"""Causal single-head attention (B=8, L=2048, D=1024, H=64) on 8 trn2 NeuronCores.

Strategy: data-parallel over batch — core b handles batch element b.
Per core (flash-attention style, S^T layout, no on-device input transposes —
the host supplies X^T pre-blocked so the contraction dim (d_model) lands on
partitions and every DMA reads long contiguous runs):

  software pipeline over q-blocks j (QB columns of Q^T):
    stream xqT/xkT/xvT column-block j from DRAM (8-16KB/partition contiguous)
    projections (column-packed: Q on PE cols 0-63, K on 64-127; V separate):
      Q^T[:,j], K^T[:,j], V^T[:,j]   (PSUM accum over 8 d-tiles)
    V^T[:,j] --PE transpose--> Vn (V natural, with an appended ones column)
    attention(j) runs with projections(j+1) interleaved to fill PE gaps:
      per k-tile pair (row-packed: even tile on PE rows 0-63, odd on 64-127,
      running concurrently since the contraction is only H=64):
        S^T[t,:] = K^T[:,t-slice].T @ Q^T[:,j-block]
        E = exp(S^T * 0.125)  (ACT over both tiles, PSUM->SBUF; diagonal
                               tiles masked by a DVE multiply)
        acc[65,QB] += Vn[t].T @ E[t]  (row 64 accumulates the softmax denom)
    PE-transpose acc -> out-natural [128,65] tiles; DVE reciprocal + scale;
    one batched DMA store per q-block.

Matmul inputs are bf16 by default (ATT_DTYPES=C; B/A trade speed for
precision via float32r); PSUM accumulation, transposes and the output path
stay fp32, and softmax needs no max-subtraction (scores are O(1) here).
"""
import os
import sys

import itertools

sys.path.insert(0, "/opt/trn_rl_repo")

_SENTINEL = object()

import ml_dtypes
import numpy as np

import concourse.bass as bass
import concourse.tile as tile
from concourse import mybir
from concourse.bass_utils import run_bass_kernel_spmd
from concourse.masks import make_identity
from bass_rust import ScopedClock, SyncInfo

B, L, D, H = 8, 2048, 1024, 64
QB = 512                 # q-block width
NQ = L // QB             # q-blocks per core
KT = QB // 128           # 128-k-tiles per q-block
ND = D // 128            # d_model tiles
NDH = ND // 2            # d tiles per DMA half-block
NCORES = 8

# Matmul-input dtype knobs. float32r = full-rate PE on 32-bit storage
# (bf16-grade products, fp32 accumulate); bfloat16 additionally halves
# DMA/SBUF traffic and doubles ACT/DVE throughput. walrus requires f32r
# matmul inputs to be *produced* as f32r, so tensors carry their dtype.
# PSUM, transposes and the output path stay f32.
_CFG = os.environ.get("ATT_DTYPES", "C")
if _CFG == "A":        # all float32r
    DTX = DTQK = DTPV = mybir.dt.float32r
elif _CFG == "B":      # bf16 activations in, f32r on-chip
    DTX = mybir.dt.bfloat16
    DTQK = DTPV = mybir.dt.float32r
else:                  # "C": bf16 everywhere
    DTX = DTQK = DTPV = mybir.dt.bfloat16
_NPX = ml_dtypes.bfloat16 if DTX == mybir.dt.bfloat16 else np.float32

# ---------------------------------------------------------------------------
# Workarounds for the container's walrus: max ONE sync-wait per instruction.
_WAIT_CAP = 1


def _patched_drain_and_barrier(self, tick_clock, wait_clock):
    drain_inst = self.nc.sync.drain()
    wait_clock.add_sem_waits(
        drain_inst.ins, ScopedClock({None: tick_clock.global_clock})
    )
    conds = list(drain_inst.ins.sync_info.on_wait)
    if len(conds) > _WAIT_CAP:
        drain_inst.ins.sync_info.on_wait.clear()
        drain_inst.ins.sync_info.on_wait.append(conds[0])
        for c in conds[1:]:
            extra = self.nc.sync.drain()
            if extra.ins.sync_info is None:
                extra.ins.sync_info = SyncInfo(on_wait=[c], on_update=[])
            else:
                extra.ins.sync_info.on_wait.append(c)
    if os.environ.get("ATT_FULL_TAIL", "0") == "1":
        self.nc.all_engine_barrier()
        assert self.sems is not None
        popped = self.nc._tile_sem_poison_stack.pop()
        assert popped is self._sem_poison
        self.nc.clear_and_free_semaphores(list(self.sems.allocated().values()))
        self.nc.all_engine_barrier()
    else:
        # the NEFF preamble re-zeroes all semaphores at the start of every
        # execution, so the expensive tail butterfly + per-sem clears are
        # redundant; just pop the bookkeeping.
        assert self.sems is not None
        popped = self.nc._tile_sem_poison_stack.pop()
        assert popped is self._sem_poison


tile.TileContext._drain_and_barrier = _patched_drain_and_barrier


def _split_excess_waits(nc):
    """Hoist overflow sem-waits onto same-engine NOPs inserted just before the
    offending instruction (engines execute their stream in order)."""
    for bb in nc.main_func.blocks:
        il = bb.instructions
        i = 0
        while i < len(il):
            ins = il[i]
            si = ins.sync_info
            if si is not None and si.on_wait and len(si.on_wait) > _WAIT_CAP:
                conds = list(si.on_wait)
                keep = conds[-_WAIT_CAP:]
                pre = conds[:-_WAIT_CAP]
                si.on_wait.clear()
                si.on_wait.extend(keep)
                nops = []
                for j in range(0, len(pre), _WAIT_CAP):
                    nop = nc.engines[ins.engine].nop().ins
                    for srcbb in nc.main_func.blocks:
                        sl = srcbb.instructions
                        if sl and sl[-1].name == nop.name:
                            sl.pop()
                            break
                    nop.sync_info = SyncInfo(
                        on_wait=list(pre[j : j + _WAIT_CAP]), on_update=[]
                    )
                    nops.append(nop)
                for k, nop in enumerate(nops):
                    il.insert(i + k, nop)
                i += len(nops)
            i += 1


# ---------------------------------------------------------------------------
def _build_nc():
    f32 = mybir.dt.float32
    nc = bass.Bass("TRN2", target_bir_lowering=False)

    # pre-blocked: xs[j, p, a, n] = X^T[a*128+p, j*QB+n] — per (block,partition)
    # the ND*QB elements are contiguous, so DMA descriptors are 16KB runs.
    xqT = nc.dram_tensor("xqT", [NQ, 128, ND, QB], DTX, kind="ExternalInput")
    xkT = nc.dram_tensor("xkT", [NQ, 128, ND, QB], DTX, kind="ExternalInput")
    xvT = nc.dram_tensor("xvT", [NQ, 128, ND, QB], DTX, kind="ExternalInput")
    # pre-swizzled: w[p, a, h] = W[a*128+p, h]
    wq = nc.dram_tensor("wq", [128, ND, H], DTX, kind="ExternalInput")
    wk = nc.dram_tensor("wk", [128, ND, H], DTX, kind="ExternalInput")
    wv = nc.dram_tensor("wv", [128, ND, H], DTX, kind="ExternalInput")
    outd = nc.dram_tensor("out", [NQ, 128, KT, H], DTPV, kind="ExternalOutput")

    with tile.TileContext(nc) as tc:
        with (
            tc.tile_pool(name="consts", bufs=1) as consts,
            tc.tile_pool(name="xin", bufs=NQ) as xin,
            tc.tile_pool(name="work", bufs=8) as work,
            tc.tile_pool(name="ps_s", bufs=2, space="PSUM") as ps_s,
            tc.tile_pool(name="ps_acc", bufs=2, space="PSUM") as ps_acc,
            tc.tile_pool(name="ps_misc", bufs=2, space="PSUM") as ps_misc,
        ):
            # constants
            wq_t = consts.tile([128, ND, H], DTX)
            wk_t = consts.tile([128, ND, H], DTX)
            wv_t = consts.tile([128, ND, H], DTX)
            nc.scalar.dma_start(out=wq_t, in_=wq[:, :, :])
            nc.scalar.dma_start(out=wk_t, in_=wk[:, :, :])
            nc.scalar.dma_start(out=wv_t, in_=wv[:, :, :])
            ident = consts.tile([128, 128], f32)
            make_identity(nc, ident)

            # QTd: Q^T duplicated on both partition halves (rhs for the two
            # row-groups of the packed S matmuls). KTt2: K^T k-tiles stored
            # even-on-partitions-0:63 / odd-on-64:127, pair t at cols t*128.
            QTd = consts.tile([128, L], DTQK)
            KTt2 = consts.tile([128, L // 2], DTQK)
            Vn = consts.tile([128, L // 128, H + 1], DTPV)
            nc.gpsimd.memset(Vn[:, :, H : H + 1], 1.0)
            # diagonal-tile masks, generated on the (otherwise idle) gpsimd:
            # maskt[p, d, f] = 1 if p + 128*d <= f else 0
            maskt = consts.tile([128, KT, QB], DTPV)
            nc.gpsimd.memset(maskt, 1.0)
            nc.gpsimd.affine_select(
                out=maskt,
                in_=maskt,
                compare_op=mybir.AluOpType.is_ge,
                fill=0.0,
                base=0,
                pattern=[[-128, KT], [1, QB]],
                channel_multiplier=-1,
            )

            def issue_block_dma(j, nchunks=2):
                """q/k chunks interleaved so the packed QK projection can
                start as soon as the first chunk pair lands; v follows."""
                xq_b = xin.tile([128, ND, QB], DTX, tag="xq")
                xk_b = xin.tile([128, ND, QB], DTX, tag="xk")
                xv_b = xin.tile([128, ND, QB], DTX, tag="xv")
                step = ND // nchunks
                for c in range(nchunks):
                    hs = slice(c * step, (c + 1) * step)
                    nc.sync.dma_start(out=xq_b[:, hs, :], in_=xqT[j, :, hs, :])
                    nc.sync.dma_start(out=xk_b[:, hs, :], in_=xkT[j, :, hs, :])
                for c in range(nchunks):
                    hs = slice(c * step, (c + 1) * step)
                    nc.sync.dma_start(out=xv_b[:, hs, :], in_=xvT[j, :, hs, :])
                return xq_b, xk_b, xv_b

            def proj_qk_gen(j, xq_b, xk_b):
                """Column-packed Q and K projections: Q on PE cols 0-63,
                K on cols 64-127, sharing the 128x128 array. A generator of
                micro-units (~1 PE pair-slot each) so the round loop can
                drizzle them into the exp-chain gaps without ever parking a
                multi-us projection burst in front of a waiting PV matmul."""
                qs = bass.ts(j, QB)
                psp = ps_misc.tile([128, QB], f32, tag="m")
                for kb in range(ND):
                    nc.tensor.matmul(
                        psp[0:64, :],
                        wq_t[:, kb, :],
                        xq_b[:, kb, :],
                        start=(kb == 0),
                        stop=(kb == ND - 1),
                        tile_position=(0, 0),
                    )
                    nc.tensor.matmul(
                        psp[64:128, :],
                        wk_t[:, kb, :],
                        xk_b[:, kb, :],
                        start=(kb == 0),
                        stop=(kb == ND - 1),
                        tile_position=(0, 64),
                    )
                    yield
                # Q^T to both partition halves of QTd. The PSUM->SBUF cast is
                # DVE-only; the SBUF->SBUF duplicate goes to the scalar
                # engine, which is always idle at the round handoff where
                # this copy gates the next round's first S matmul.
                nc.vector.tensor_copy(QTd[0:64, qs], psp[0:64, :])
                yield
                nc.scalar.copy(QTd[64:128, qs], QTd[0:64, qs])
                yield
                # K^T k-tiles by parity: block j covers k-tiles 4j..4j+3,
                # i.e. pair-columns 2j and 2j+1
                pcols = bass.ds(2 * j * 128, 256)
                nc.vector.tensor_copy(
                    KTt2[0:64, pcols],
                    psp[64:128, :].rearrange("p (a c) -> p a c", c=128)[
                        :, 0::2, :
                    ],
                )
                yield
                nc.vector.tensor_copy(
                    KTt2[64:128, pcols],
                    psp[64:128, :].rearrange("p (a c) -> p a c", c=128)[
                        :, 1::2, :
                    ],
                )
                yield

            def proj_v_gen(j, x_b):
                psp = ps_misc.tile([128, QB], f32, tag="m")
                for kb in range(ND):
                    nc.tensor.matmul(
                        psp[0:64, :],
                        wv_t[:, kb, :],
                        x_b[:, kb, :],
                        start=(kb == 0),
                        stop=(kb == ND - 1),
                    )
                    yield
                vts = work.tile([64, QB], f32, tag="vts")
                nc.vector.tensor_copy(vts, psp[0:64, :])
                yield
                # all four V^T->V PE transposes land in ONE psum tile so a
                # single fused DVE copy moves them into Vn
                psm = ps_misc.tile([128, QB], f32, tag="m")
                pst = psm[:, 0 : KT * (H + 1)].rearrange(
                    "p (t c) -> p t c", c=H + 1
                )
                for t4 in range(KT):
                    nc.tensor.transpose(
                        pst[:, t4, 0:H], vts[:, bass.ts(t4, 128)], ident[0:H, 0:H]
                    )
                    yield
                nc.vector.tensor_copy(
                    Vn[:, j * KT : (j + 1) * KT, 0:H], pst[:, :, 0:H]
                )
                yield

            def s_exp_pair(j, tp):
                """S^T for k-tile pair tp of q-block j, then exp (+ diagonal
                mask on DVE). Returns the exps tile for the later PV step."""
                qs = bass.ts(j, QB)
                pss = ps_s.tile([128, 2, QB], f32, tag="s")
                exps = work.tile([128, 2, QB], DTPV, tag="exps")
                # row-packed: even k-tile on PE rows 0-63, odd on 64-127,
                # running concurrently (K=64 each)
                nc.tensor.matmul(
                    pss[:, 0, :],
                    KTt2[0:64, bass.ts(tp, 128)],
                    QTd[0:64, qs],
                    start=True,
                    stop=True,
                    tile_position=(0, 0),
                )
                nc.tensor.matmul(
                    pss[:, 1, :],
                    KTt2[64:128, bass.ts(tp, 128)],
                    QTd[64:128, qs],
                    start=True,
                    stop=True,
                    tile_position=(64, 0),
                )
                nc.scalar.activation(
                    exps, pss, mybir.ActivationFunctionType.Exp, scale=0.125
                )
                d0 = 2 * tp - j * KT
                if d0 >= 0:
                    # both k-tiles of a diagonal pair are diagonal: one fused
                    # DVE multiply over [128, 2, QB]
                    nc.vector.tensor_mul(exps, exps, maskt[:, d0 : d0 + 2, :])
                return exps

            def pv_pair(j, tp, acc, exps, first, last):
                for i in range(2):
                    t = 2 * tp + i
                    nc.tensor.matmul(
                        acc,
                        Vn[:, t, :],
                        exps[:, i, :],
                        start=(first and i == 0),
                        stop=(last and i == 1),
                    )

            def finalize_block(j, acc):
                oT = work.tile([H + 1, QB], f32, tag="oT")
                nc.vector.tensor_copy(oT, acc)
                obuf = work.tile([128, KT, H], DTPV, tag="obuf")
                psm = ps_misc.tile([128, QB], f32, tag="m")
                pso = psm[:, 0 : KT * (H + 1)].rearrange(
                    "p (t c) -> p t c", c=H + 1
                )
                for t4 in range(KT):
                    nc.tensor.transpose(
                        pso[:, t4, :],
                        oT[:, bass.ts(t4, 128)],
                        ident[0 : H + 1, 0 : H + 1],
                    )
                rcp = work.tile([128, KT], f32, tag="rcp")
                nc.vector.reciprocal(rcp, pso[:, :, H])
                for t4 in range(KT):
                    nc.vector.tensor_scalar_mul(
                        obuf[:, t4, :], pso[:, t4, 0:H], rcp[:, t4 : t4 + 1]
                    )
                nc.sync.dma_start(out=outd[j, :, :, :], in_=obuf)

            # ---- software pipeline: ALL x-blocks prefetched up front (they
            # all fit in SBUF), so the 16 DMA queues stream flat-out from t=0
            # instead of bursting just-in-time. proj(0) first, then round j
            # runs attention(j) one S-pair AHEAD of its PV consumer, with
            # proj(j+1) micro-units drained into the gaps so the PE stream
            # stays dense (warm HAM) while the scalar exp chain runs.
            xq_b, xk_b, xv_b = issue_block_dma(0, nchunks=4)
            pending = {}
            for jj in range(1, NQ):
                pending[jj] = issue_block_dma(jj, nchunks=1)
            # a short PE warmup toward HAM un-throttle while block 0 lands
            for _ in range(4):
                dum = ps_misc.tile([128, QB], f32, tag="m")
                nc.tensor.matmul(
                    dum[:, 0 : H + 1],
                    ident,
                    ident[:, 0 : H + 1],
                    start=True,
                    stop=True,
                )
            for _ in proj_qk_gen(0, xq_b, xk_b):
                pass
            for _ in proj_v_gen(0, xv_b):
                pass
            # two S/exp pairs run AHEAD of their PV consumers, so the scalar
            # exp chain never waits behind a PV that is itself waiting on the
            # previous exp. Each round's LAST two PVs are carried across the
            # round boundary and flushed between the next round's S emissions
            # — so a PV stalled on exp or V-data never blocks the next
            # round's S matmuls in the in-order PE queue. Diagonal (masked)
            # pairs are processed FIRST so the exp->mask->PV chain sits at
            # the round head (where carried work fills the PE) instead of
            # the tail. The next round's first S/exp is hoisted into this
            # round's epilogue, after the proj units that produce its QTd.
            fin = None  # deferred (j, acc) finalize
            head = None  # hoisted (tp, exps) of next round
            carry = []  # trailing PVs: (j, tp, acc, exps, first, last)
            CARRY = 2
            for j in range(NQ):
                units = iter(())
                n_units = 0
                if j + 1 < NQ:
                    nxq, nxk, nxv = pending.pop(j + 1)
                    units = itertools.chain(
                        proj_qk_gen(j + 1, nxq, nxk), proj_v_gen(j + 1, nxv)
                    )
                    n_units = 2 * ND + 4 + 2 + KT
                acc = ps_acc.tile([H + 1, QB], f32)
                nkt = (j + 1) * KT
                npairs = nkt // 2
                order = list(range(npairs))
                if npairs > 2:
                    order = order[-2:] + order[:-2]  # diagonal pairs first
                quota = -(-n_units // max(1, 2 * npairs - 1))  # ceil

                def drain(n):
                    for _ in range(n):
                        if next(units, _SENTINEL) is _SENTINEL:
                            return

                inflight = []
                if head is not None:
                    inflight.append(head)
                    head = None
                pv_seq = 0
                for oi in range(len(inflight), npairs):
                    inflight.append((order[oi], s_exp_pair(j, order[oi])))
                    if carry:
                        pv_pair(*carry.pop(0))
                        if not carry and fin is not None:
                            finalize_block(*fin)
                            fin = None
                    elif fin is not None:
                        finalize_block(*fin)
                        fin = None
                    drain(quota)
                    if len(inflight) > 2:
                        ptp, pexps = inflight.pop(0)
                        pv_pair(
                            j, ptp, acc, pexps, pv_seq == 0,
                            pv_seq == npairs - 1,
                        )
                        pv_seq += 1
                keep = CARRY if j + 1 < NQ else 0
                while len(inflight) > keep:
                    drain(quota)
                    ptp, pexps = inflight.pop(0)
                    pv_pair(
                        j, ptp, acc, pexps, pv_seq == 0, pv_seq == npairs - 1
                    )
                    pv_seq += 1
                drain(n_units)  # proj leftovers MUST precede the hoisted S
                # (Tile deps follow emission order: it reads QTd(j+1))
                assert not carry
                for ptp, pexps in inflight:
                    carry.append(
                        (j, ptp, acc, pexps, pv_seq == 0, pv_seq == npairs - 1)
                    )
                    pv_seq += 1
                if j + 1 < NQ:
                    nx_np = (j + 2) * KT // 2
                    tp0 = nx_np - 2 if nx_np > 2 else 0
                    head = (tp0, s_exp_pair(j + 1, tp0))
                fin = (j, acc)
            assert not carry
            finalize_block(*fin)

    _split_excess_waits(nc)
    return nc


_NC = None


def _get_nc():
    global _NC
    if _NC is None:
        _NC = _build_nc()
    return _NC


def _np_of(dt):
    return np.dtype(ml_dtypes.bfloat16) if dt == mybir.dt.bfloat16 else np.float32


def _block_x(x):
    """[L, D] activations -> [NQ, 128, ND, QB] pre-blocked X^T."""
    # xs[j, p, a, n] = X^T[a*128+p, j*QB+n] = x[j*QB+n, a*128+p]
    return np.ascontiguousarray(
        np.asarray(x, np.float32)
        .reshape(NQ, QB, ND, 128)
        .transpose(0, 3, 2, 1)
        .astype(_NPX)
    )


def _swizzle_w(w):
    """[D, H] -> [128, ND, H]: w[p, a, h] = W[a*128+p, h]."""
    return np.ascontiguousarray(
        np.asarray(w, np.float32)
        .reshape(ND, 128, H)
        .transpose(1, 0, 2)
        .astype(_NPX)
    )


def make_in_maps(inputs):
    """Build per-core in_maps from a reference-style inputs dict."""
    wq = _swizzle_w(inputs["Wq"])
    wk = _swizzle_w(inputs["Wk"])
    wv = _swizzle_w(inputs["Wv"])
    return [
        {
            "xqT": _block_x(inputs["idx_q"][b]),
            "xkT": _block_x(inputs["idx_k"][b]),
            "xvT": _block_x(inputs["idx_v"][b]),
            "wq": wq,
            "wk": wk,
            "wv": wv,
        }
        for b in range(NCORES)
    ]


def kernel(idx_k, idx_q, idx_v, msk, Wk, Wq, Wv, **_unused):
    in_maps = make_in_maps(
        {
            "idx_k": idx_k,
            "idx_q": idx_q,
            "idx_v": idx_v,
            "Wk": Wk,
            "Wq": Wq,
            "Wv": Wv,
        }
    )
    nc = _get_nc()
    res = run_bass_kernel_spmd(nc, in_maps, core_ids=list(range(NCORES)))
    return np.stack(
        [
            res.results[b]["out"]
            .astype(np.float32)
            .transpose(0, 2, 1, 3)
            .reshape(L, H)
            for b in range(NCORES)
        ],
        axis=0,
    )


def run_traced(in_maps, tmpdir="/tmp/att_trace", trace_cores=None):
    """Test-harness helper: run with NTFF tracing, return BassKernelResults."""
    import os
    import shutil

    shutil.rmtree(tmpdir, ignore_errors=True)
    os.makedirs(tmpdir)
    return run_bass_kernel_spmd(
        _get_nc(),
        in_maps,
        core_ids=list(range(NCORES)),
        trace=True,
        tmpdir=tmpdir,
        trace_cores=trace_cores,
    )



# revision 52
# speedup vs baseline: 1.0466x; 1.0466x over previous
"""Causal single-head attention (B=8, L=2048, D=1024, H=64) on 8 trn2 NeuronCores.

Strategy: data-parallel over batch — core b handles batch element b.
Per core (flash-attention style, S^T layout, no on-device input transposes —
the host supplies X^T pre-blocked so the contraction dim (d_model) lands on
partitions and every DMA reads long contiguous runs):

  software pipeline over q-blocks j (QB columns of Q^T):
    stream xqT/xkT/xvT column-block j from DRAM (8-16KB/partition contiguous)
    projections (column-packed: Q on PE cols 0-63, K on 64-127; V separate):
      Q^T[:,j], K^T[:,j], V^T[:,j]   (PSUM accum over 8 d-tiles)
    V^T[:,j] --PE transpose--> Vn (V natural, with an appended ones column)
    attention(j) runs with projections(j+1) interleaved to fill PE gaps:
      per k-tile pair (row-packed: even tile on PE rows 0-63, odd on 64-127,
      running concurrently since the contraction is only H=64):
        S^T[t,:] = K^T[:,t-slice].T @ Q^T[:,j-block]
        E = exp(S^T * 0.125)  (ACT over both tiles, PSUM->SBUF; diagonal
                               tiles masked by a DVE multiply)
        acc[65,QB] += Vn[t].T @ E[t]  (row 64 accumulates the softmax denom)
    PE-transpose acc -> out-natural [128,65] tiles; DVE reciprocal + scale;
    one batched DMA store per q-block.

Matmul inputs are bf16 by default (ATT_DTYPES=C; B/A trade speed for
precision via float32r); PSUM accumulation, transposes and the output path
stay fp32, and softmax needs no max-subtraction (scores are O(1) here).
"""
import os
import sys

import itertools

sys.path.insert(0, "/opt/trn_rl_repo")

_SENTINEL = object()

import ml_dtypes
import numpy as np

import concourse.bass as bass
import concourse.tile as tile
from concourse import mybir
from concourse.bass_utils import run_bass_kernel_spmd
from concourse.masks import make_identity
from bass_rust import ScopedClock, SyncInfo

B, L, D, H = 8, 2048, 1024, 64
QB = 512                 # q-block width
NQ = L // QB             # q-blocks per core
KT = QB // 128           # 128-k-tiles per q-block
ND = D // 128            # d_model tiles
NDH = ND // 2            # d tiles per DMA half-block
NCORES = 8

# Matmul-input dtype knobs. float32r = full-rate PE on 32-bit storage
# (bf16-grade products, fp32 accumulate); bfloat16 additionally halves
# DMA/SBUF traffic and doubles ACT/DVE throughput. walrus requires f32r
# matmul inputs to be *produced* as f32r, so tensors carry their dtype.
# PSUM, transposes and the output path stay f32.
_CFG = os.environ.get("ATT_DTYPES", "C")
if _CFG == "A":        # all float32r
    DTX = DTQK = DTPV = mybir.dt.float32r
elif _CFG == "B":      # bf16 activations in, f32r on-chip
    DTX = mybir.dt.bfloat16
    DTQK = DTPV = mybir.dt.float32r
else:                  # "C": bf16 everywhere
    DTX = DTQK = DTPV = mybir.dt.bfloat16
_NPX = ml_dtypes.bfloat16 if DTX == mybir.dt.bfloat16 else np.float32

# ---------------------------------------------------------------------------
# Workarounds for the container's walrus: max ONE sync-wait per instruction.
_WAIT_CAP = 1


def _patched_drain_and_barrier(self, tick_clock, wait_clock):
    drain_inst = self.nc.sync.drain()
    wait_clock.add_sem_waits(
        drain_inst.ins, ScopedClock({None: tick_clock.global_clock})
    )
    conds = list(drain_inst.ins.sync_info.on_wait)
    if len(conds) > _WAIT_CAP:
        drain_inst.ins.sync_info.on_wait.clear()
        drain_inst.ins.sync_info.on_wait.append(conds[0])
        for c in conds[1:]:
            extra = self.nc.sync.drain()
            if extra.ins.sync_info is None:
                extra.ins.sync_info = SyncInfo(on_wait=[c], on_update=[])
            else:
                extra.ins.sync_info.on_wait.append(c)
    if os.environ.get("ATT_FULL_TAIL", "0") == "1":
        self.nc.all_engine_barrier()
        assert self.sems is not None
        popped = self.nc._tile_sem_poison_stack.pop()
        assert popped is self._sem_poison
        self.nc.clear_and_free_semaphores(list(self.sems.allocated().values()))
        self.nc.all_engine_barrier()
    else:
        # the NEFF preamble re-zeroes all semaphores at the start of every
        # execution, so the expensive tail butterfly + per-sem clears are
        # redundant; just pop the bookkeeping.
        assert self.sems is not None
        popped = self.nc._tile_sem_poison_stack.pop()
        assert popped is self._sem_poison


tile.TileContext._drain_and_barrier = _patched_drain_and_barrier


def _split_excess_waits(nc):
    """Hoist overflow sem-waits onto same-engine NOPs inserted just before the
    offending instruction (engines execute their stream in order)."""
    for bb in nc.main_func.blocks:
        il = bb.instructions
        i = 0
        while i < len(il):
            ins = il[i]
            si = ins.sync_info
            if si is not None and si.on_wait and len(si.on_wait) > _WAIT_CAP:
                conds = list(si.on_wait)
                keep = conds[-_WAIT_CAP:]
                pre = conds[:-_WAIT_CAP]
                si.on_wait.clear()
                si.on_wait.extend(keep)
                nops = []
                for j in range(0, len(pre), _WAIT_CAP):
                    nop = nc.engines[ins.engine].nop().ins
                    for srcbb in nc.main_func.blocks:
                        sl = srcbb.instructions
                        if sl and sl[-1].name == nop.name:
                            sl.pop()
                            break
                    nop.sync_info = SyncInfo(
                        on_wait=list(pre[j : j + _WAIT_CAP]), on_update=[]
                    )
                    nops.append(nop)
                for k, nop in enumerate(nops):
                    il.insert(i + k, nop)
                i += len(nops)
            i += 1


# ---------------------------------------------------------------------------
def _build_nc():
    f32 = mybir.dt.float32
    nc = bass.Bass("TRN2", target_bir_lowering=False)

    # pre-blocked: xs[j, p, a, n] = X^T[a*128+p, j*QB+n] — per (block,partition)
    # the ND*QB elements are contiguous, so DMA descriptors are 16KB runs.
    xqT = nc.dram_tensor("xqT", [NQ, 128, ND, QB], DTX, kind="ExternalInput")
    xkT = nc.dram_tensor("xkT", [NQ, 128, ND, QB], DTX, kind="ExternalInput")
    xvT = nc.dram_tensor("xvT", [NQ, 128, ND, QB], DTX, kind="ExternalInput")
    # pre-swizzled: w[p, a, h] = W[a*128+p, h]
    wq = nc.dram_tensor("wq", [128, ND, H], DTX, kind="ExternalInput")
    wk = nc.dram_tensor("wk", [128, ND, H], DTX, kind="ExternalInput")
    wv = nc.dram_tensor("wv", [128, ND, H], DTX, kind="ExternalInput")
    outd = nc.dram_tensor("out", [NQ, 128, KT, H], DTPV, kind="ExternalOutput")

    with tile.TileContext(nc) as tc:
        with (
            tc.tile_pool(name="consts", bufs=1) as consts,
            tc.tile_pool(name="xin", bufs=NQ) as xin,
            tc.tile_pool(name="work", bufs=8) as work,
            tc.tile_pool(name="ps_s", bufs=2, space="PSUM") as ps_s,
            tc.tile_pool(name="ps_acc", bufs=2, space="PSUM") as ps_acc,
            tc.tile_pool(name="ps_misc", bufs=2, space="PSUM") as ps_misc,
        ):
            # constants
            wq_t = consts.tile([128, ND, H], DTX)
            wk_t = consts.tile([128, ND, H], DTX)
            wv_t = consts.tile([128, ND, H], DTX)
            nc.scalar.dma_start(out=wq_t, in_=wq[:, :, :])
            nc.scalar.dma_start(out=wk_t, in_=wk[:, :, :])
            nc.scalar.dma_start(out=wv_t, in_=wv[:, :, :])
            ident = consts.tile([128, 128], f32)
            make_identity(nc, ident)

            # QTd: Q^T duplicated on both partition halves (rhs for the two
            # row-groups of the packed S matmuls). KTt2: K^T k-tiles stored
            # even-on-partitions-0:63 / odd-on-64:127, pair t at cols t*128.
            QTd = consts.tile([128, L], DTQK)
            KTt2 = consts.tile([128, L // 2], DTQK)
            Vn = consts.tile([128, L // 128, H + 1], DTPV)
            nc.gpsimd.memset(Vn[:, :, H : H + 1], 1.0)
            # diagonal-tile masks, generated on the (otherwise idle) gpsimd:
            # maskt[p, d, f] = 1 if p + 128*d <= f else 0
            maskt = consts.tile([128, KT, QB], DTPV)
            nc.gpsimd.memset(maskt, 1.0)
            nc.gpsimd.affine_select(
                out=maskt,
                in_=maskt,
                compare_op=mybir.AluOpType.is_ge,
                fill=0.0,
                base=0,
                pattern=[[-128, KT], [1, QB]],
                channel_multiplier=-1,
            )

            def issue_block_dma(j, nchunks=2):
                """q/k chunks interleaved so the packed QK projection can
                start as soon as the first chunk pair lands; v follows."""
                xq_b = xin.tile([128, ND, QB], DTX, tag="xq")
                xk_b = xin.tile([128, ND, QB], DTX, tag="xk")
                xv_b = xin.tile([128, ND, QB], DTX, tag="xv")
                step = ND // nchunks
                for c in range(nchunks):
                    hs = slice(c * step, (c + 1) * step)
                    nc.sync.dma_start(out=xq_b[:, hs, :], in_=xqT[j, :, hs, :])
                    nc.sync.dma_start(out=xk_b[:, hs, :], in_=xkT[j, :, hs, :])
                for c in range(nchunks):
                    hs = slice(c * step, (c + 1) * step)
                    nc.sync.dma_start(out=xv_b[:, hs, :], in_=xvT[j, :, hs, :])
                return xq_b, xk_b, xv_b

            def proj_qk_gen(j, xq_b, xk_b):
                """Column-packed Q and K projections: Q on PE cols 0-63,
                K on cols 64-127, sharing the 128x128 array. A generator of
                micro-units (~1 PE pair-slot each) so the round loop can
                drizzle them into the exp-chain gaps without ever parking a
                multi-us projection burst in front of a waiting PV matmul."""
                qs = bass.ts(j, QB)
                psp = ps_misc.tile([128, QB], f32, tag="m")
                for kb in range(ND):
                    nc.tensor.matmul(
                        psp[0:64, :],
                        wq_t[:, kb, :],
                        xq_b[:, kb, :],
                        start=(kb == 0),
                        stop=(kb == ND - 1),
                        tile_position=(0, 0),
                    )
                    nc.tensor.matmul(
                        psp[64:128, :],
                        wk_t[:, kb, :],
                        xk_b[:, kb, :],
                        start=(kb == 0),
                        stop=(kb == ND - 1),
                        tile_position=(0, 64),
                    )
                    yield
                # Q^T to both partition halves of QTd. The PSUM->SBUF cast is
                # DVE-only; the SBUF->SBUF duplicate goes to the scalar
                # engine, which is always idle at the round handoff where
                # this copy gates the next round's first S matmul.
                nc.vector.tensor_copy(QTd[0:64, qs], psp[0:64, :])
                yield
                nc.scalar.copy(QTd[64:128, qs], QTd[0:64, qs])
                yield
                # K^T k-tiles by parity: block j covers k-tiles 4j..4j+3,
                # i.e. pair-columns 2j and 2j+1
                pcols = bass.ds(2 * j * 128, 256)
                nc.vector.tensor_copy(
                    KTt2[0:64, pcols],
                    psp[64:128, :].rearrange("p (a c) -> p a c", c=128)[
                        :, 0::2, :
                    ],
                )
                yield
                nc.vector.tensor_copy(
                    KTt2[64:128, pcols],
                    psp[64:128, :].rearrange("p (a c) -> p a c", c=128)[
                        :, 1::2, :
                    ],
                )
                yield

            def proj_v_gen(j, x_b):
                psp = ps_misc.tile([128, QB], f32, tag="m")
                for kb in range(ND):
                    nc.tensor.matmul(
                        psp[0:64, :],
                        wv_t[:, kb, :],
                        x_b[:, kb, :],
                        start=(kb == 0),
                        stop=(kb == ND - 1),
                    )
                    yield
                vts = work.tile([64, QB], f32, tag="vts")
                nc.vector.tensor_copy(vts, psp[0:64, :])
                yield
                # all four V^T->V PE transposes land in ONE psum tile so a
                # single fused DVE copy moves them into Vn
                psm = ps_misc.tile([128, QB], f32, tag="m")
                pst = psm[:, 0 : KT * (H + 1)].rearrange(
                    "p (t c) -> p t c", c=H + 1
                )
                for t4 in range(KT):
                    nc.tensor.transpose(
                        pst[:, t4, 0:H], vts[:, bass.ts(t4, 128)], ident[0:H, 0:H]
                    )
                    yield
                nc.vector.tensor_copy(
                    Vn[:, j * KT : (j + 1) * KT, 0:H], pst[:, :, 0:H]
                )
                yield

            def s_exp_pair(j, tp):
                """S^T for k-tile pair tp of q-block j, then exp (+ diagonal
                mask on DVE). Returns the exps tile for the later PV step."""
                qs = bass.ts(j, QB)
                pss = ps_s.tile([128, 2, QB], f32, tag="s")
                exps = work.tile([128, 2, QB], DTPV, tag="exps")
                # row-packed: even k-tile on PE rows 0-63, odd on 64-127,
                # running concurrently (K=64 each)
                nc.tensor.matmul(
                    pss[:, 0, :],
                    KTt2[0:64, bass.ts(tp, 128)],
                    QTd[0:64, qs],
                    start=True,
                    stop=True,
                    tile_position=(0, 0),
                )
                nc.tensor.matmul(
                    pss[:, 1, :],
                    KTt2[64:128, bass.ts(tp, 128)],
                    QTd[64:128, qs],
                    start=True,
                    stop=True,
                    tile_position=(64, 0),
                )
                nc.scalar.activation(
                    exps, pss, mybir.ActivationFunctionType.Exp, scale=0.125
                )
                d0 = 2 * tp - j * KT
                if d0 >= 0:
                    # both k-tiles of a diagonal pair are diagonal: one fused
                    # DVE multiply over [128, 2, QB]
                    nc.vector.tensor_mul(exps, exps, maskt[:, d0 : d0 + 2, :])
                return exps

            def pv_pair(j, tp, acc, exps, first, last):
                for i in range(2):
                    t = 2 * tp + i
                    nc.tensor.matmul(
                        acc,
                        Vn[:, t, :],
                        exps[:, i, :],
                        start=(first and i == 0),
                        stop=(last and i == 1),
                    )

            def finalize_block(j, acc):
                oT = work.tile([H + 1, QB], f32, tag="oT")
                nc.vector.tensor_copy(oT, acc)
                obuf = work.tile([128, KT, H], DTPV, tag="obuf")
                psm = ps_misc.tile([128, QB], f32, tag="m")
                pso = psm[:, 0 : KT * (H + 1)].rearrange(
                    "p (t c) -> p t c", c=H + 1
                )
                for t4 in range(KT):
                    nc.tensor.transpose(
                        pso[:, t4, :],
                        oT[:, bass.ts(t4, 128)],
                        ident[0 : H + 1, 0 : H + 1],
                    )
                rcp = work.tile([128, KT], f32, tag="rcp")
                nc.vector.reciprocal(rcp, pso[:, :, H])
                for t4 in range(KT):
                    nc.vector.tensor_scalar_mul(
                        obuf[:, t4, :], pso[:, t4, 0:H], rcp[:, t4 : t4 + 1]
                    )
                nc.sync.dma_start(out=outd[j, :, :, :], in_=obuf)

            # ---- software pipeline: ALL x-blocks prefetched up front (they
            # all fit in SBUF), so the 16 DMA queues stream flat-out from t=0
            # instead of bursting just-in-time. proj(0) first, then round j
            # runs attention(j) one S-pair AHEAD of its PV consumer, with
            # proj(j+1) micro-units drained into the gaps so the PE stream
            # stays dense (warm HAM) while the scalar exp chain runs.
            xq_b, xk_b, xv_b = issue_block_dma(0, nchunks=2)
            pending = {}
            for jj in range(1, NQ):
                pending[jj] = issue_block_dma(jj, nchunks=1)
            # a short PE warmup toward HAM un-throttle while block 0 lands
            for _ in range(4):
                dum = ps_misc.tile([128, QB], f32, tag="m")
                nc.tensor.matmul(
                    dum[:, 0 : H + 1],
                    ident,
                    ident[:, 0 : H + 1],
                    start=True,
                    stop=True,
                )
            for _ in proj_qk_gen(0, xq_b, xk_b):
                pass
            for _ in proj_v_gen(0, xv_b):
                pass
            # two S/exp pairs run AHEAD of their PV consumers, so the scalar
            # exp chain never waits behind a PV that is itself waiting on the
            # previous exp. Each round's LAST two PVs are carried across the
            # round boundary and flushed between the next round's S emissions
            # — so a PV stalled on exp or V-data never blocks the next
            # round's S matmuls in the in-order PE queue. Diagonal (masked)
            # pairs are processed FIRST so the exp->mask->PV chain sits at
            # the round head (where carried work fills the PE) instead of
            # the tail. The next round's first S/exp is hoisted into this
            # round's epilogue, after the proj units that produce its QTd.
            fin = None  # deferred (j, acc) finalize
            head = None  # hoisted (tp, exps) of next round
            carry = []  # trailing PVs: (j, tp, acc, exps, first, last)
            CARRY = 2
            for j in range(NQ):
                units = iter(())
                n_units = 0
                if j + 1 < NQ:
                    nxq, nxk, nxv = pending.pop(j + 1)
                    units = itertools.chain(
                        proj_qk_gen(j + 1, nxq, nxk), proj_v_gen(j + 1, nxv)
                    )
                    n_units = 2 * ND + 4 + 2 + KT
                acc = ps_acc.tile([H + 1, QB], f32)
                nkt = (j + 1) * KT
                npairs = nkt // 2
                order = list(range(npairs))
                if npairs > 2:
                    order = order[-2:] + order[:-2]  # diagonal pairs first
                quota = -(-n_units // max(1, 2 * npairs - 1))  # ceil

                def drain(n):
                    for _ in range(n):
                        if next(units, _SENTINEL) is _SENTINEL:
                            return

                inflight = []
                if head is not None:
                    inflight.append(head)
                    head = None
                pv_seq = 0
                for oi in range(len(inflight), npairs):
                    inflight.append((order[oi], s_exp_pair(j, order[oi])))
                    if carry:
                        pv_pair(*carry.pop(0))
                        if not carry and fin is not None:
                            finalize_block(*fin)
                            fin = None
                    elif fin is not None:
                        finalize_block(*fin)
                        fin = None
                    drain(quota)
                    if len(inflight) > 2:
                        ptp, pexps = inflight.pop(0)
                        pv_pair(
                            j, ptp, acc, pexps, pv_seq == 0,
                            pv_seq == npairs - 1,
                        )
                        pv_seq += 1
                keep = CARRY if j + 1 < NQ else 0
                while len(inflight) > keep:
                    drain(quota)
                    ptp, pexps = inflight.pop(0)
                    pv_pair(
                        j, ptp, acc, pexps, pv_seq == 0, pv_seq == npairs - 1
                    )
                    pv_seq += 1
                drain(n_units)  # proj leftovers MUST precede the hoisted S
                # (Tile deps follow emission order: it reads QTd(j+1))
                assert not carry
                for ptp, pexps in inflight:
                    carry.append(
                        (j, ptp, acc, pexps, pv_seq == 0, pv_seq == npairs - 1)
                    )
                    pv_seq += 1
                if j + 1 < NQ:
                    nx_np = (j + 2) * KT // 2
                    tp0 = nx_np - 2 if nx_np > 2 else 0
                    head = (tp0, s_exp_pair(j + 1, tp0))
                fin = (j, acc)
            assert not carry
            finalize_block(*fin)

    _split_excess_waits(nc)
    return nc


_NC = None


def _get_nc():
    global _NC
    if _NC is None:
        _NC = _build_nc()
    return _NC


def _np_of(dt):
    return np.dtype(ml_dtypes.bfloat16) if dt == mybir.dt.bfloat16 else np.float32


def _block_x(x):
    """[L, D] activations -> [NQ, 128, ND, QB] pre-blocked X^T."""
    # xs[j, p, a, n] = X^T[a*128+p, j*QB+n] = x[j*QB+n, a*128+p]
    return np.ascontiguousarray(
        np.asarray(x, np.float32)
        .reshape(NQ, QB, ND, 128)
        .transpose(0, 3, 2, 1)
        .astype(_NPX)
    )


def _swizzle_w(w):
    """[D, H] -> [128, ND, H]: w[p, a, h] = W[a*128+p, h]."""
    return np.ascontiguousarray(
        np.asarray(w, np.float32)
        .reshape(ND, 128, H)
        .transpose(1, 0, 2)
        .astype(_NPX)
    )


def make_in_maps(inputs):
    """Build per-core in_maps from a reference-style inputs dict."""
    wq = _swizzle_w(inputs["Wq"])
    wk = _swizzle_w(inputs["Wk"])
    wv = _swizzle_w(inputs["Wv"])
    return [
        {
            "xqT": _block_x(inputs["idx_q"][b]),
            "xkT": _block_x(inputs["idx_k"][b]),
            "xvT": _block_x(inputs["idx_v"][b]),
            "wq": wq,
            "wk": wk,
            "wv": wv,
        }
        for b in range(NCORES)
    ]


def kernel(idx_k, idx_q, idx_v, msk, Wk, Wq, Wv, **_unused):
    in_maps = make_in_maps(
        {
            "idx_k": idx_k,
            "idx_q": idx_q,
            "idx_v": idx_v,
            "Wk": Wk,
            "Wq": Wq,
            "Wv": Wv,
        }
    )
    nc = _get_nc()
    res = run_bass_kernel_spmd(nc, in_maps, core_ids=list(range(NCORES)))
    return np.stack(
        [
            res.results[b]["out"]
            .astype(np.float32)
            .transpose(0, 2, 1, 3)
            .reshape(L, H)
            for b in range(NCORES)
        ],
        axis=0,
    )


def run_traced(in_maps, tmpdir="/tmp/att_trace", trace_cores=None):
    """Test-harness helper: run with NTFF tracing, return BassKernelResults."""
    import os
    import shutil

    shutil.rmtree(tmpdir, ignore_errors=True)
    os.makedirs(tmpdir)
    return run_bass_kernel_spmd(
        _get_nc(),
        in_maps,
        core_ids=list(range(NCORES)),
        trace=True,
        tmpdir=tmpdir,
        trace_cores=trace_cores,
    )

